# revision 64
# baseline (speedup 1.0000x reference)
"""AttentionRope TRN2 kernel: data-parallel over batch (1 batch elem / core).

v2 redesign vs baseline:
  - tokens padded 1025 -> 1152 (9 full 128-tiles); pad k-tokens neutralized
    by zeroing their [v|1] rows (no numerator/denominator contribution),
    pad q-tokens never stored.
  - QK for each (head, k-tile) lands in one 3-bank [128,1152] PSUM tile ->
    ONE exp activation per k-tile (scale folded).
  - PV accumulates [v|1]^T @ expS into psum rows 0..64; softmax denominator
    reciprocal via reciprocal_approx_fast on row 64; ones-outer-product
    (f32r matmul, tile_position col=64) broadcasts recip into rows 64..127
    of the SAME psum bank; one fused DVE mult writes oT.
  - LN+rope: batched stats via X-axis reduces + broadcast-view normalize;
    rope via premultiplied (w*cos, swap(w)*sin) tables and two strided
    even/odd combines (no separate rotate pass). Work split DVE/GpSimd/ACT.
  - biases (all zero in practice) added via K=1 f32r matmuls into psum.
  - software pipelining: QK(h+1) emitted before PV(h); proj output DMA'd
    from PSUM.
"""

import os
import numpy as np
from contextlib import ExitStack

import concourse.bass as bass
from concourse import bacc as _bacc
import concourse.mybir as mybir
import concourse.tile as tile
from concourse.bass_utils import run_bass_kernel_spmd
from concourse.masks import make_identity

B, NT, C = 8, 1025, 1024
H, HD = 16, 64
EPS = 1e-6
SCALE = HD ** -0.5
P = 128
NTT = 9                 # token tiles
NTOK = NTT * P          # padded token count 1152
F32 = mybir.dt.float32
F32R = mybir.dt.float32r
BF16 = mybir.dt.bfloat16
AF = mybir.ActivationFunctionType
ALU = mybir.AluOpType
AXM = mybir.AxisListType

LAST_RESULT = None

# q chunks: (q0, qn, psum col offset). chunk 3 overlaps tokens 897..1024 so
# every computed q column is a real token.
QCH = [(0, 512, 0), (512, 512, 512), (897, 128, 1024)]


def _r(ap):
    return ap.bitcast(F32R)


def build_kernel(ctx, tc, X, ROPE, QKVW, QKVB, QNW, QNB, KNW, KNB, PW, PB, OUT,
                 has_qkv_b, has_qn_b, has_kn_b, has_pj_b):
    NPH = int(os.environ.get("KPHASES", "9"))
    nc = tc.nc

    consts = ctx.enter_context(tc.tile_pool(name="consts", bufs=1))
    arena = ctx.enter_context(tc.tile_pool(name="arena", bufs=1))

    ident0 = consts.tile([P, P], F32, tag="ident0")
    make_identity(nc, ident0)
    ident = consts.tile([P, P], BF16, tag="ident")
    nc.vector.tensor_copy(ident, ident0)
    ones64 = consts.tile([1, HD], F32, tag="ones64")
    nc.vector.memset(ones64, 1.0)
    ones_col = consts.tile([1, HD], F32, tag="ones_col")
    nc.vector.tensor_copy(_r(ones_col), ones64)
    ones_row = consts.tile([1, P], F32, tag="ones_row")
    nc.vector.memset(ones_row, 1.0)
    ones_row_r = consts.tile([1, P], F32, tag="ones_row_r")
    nc.vector.tensor_copy(_r(ones_row_r), ones_row)
    eps_t = consts.tile([P, 1], F32)
    nc.vector.memset(eps_t, EPS)

    def rep_tile(dram, w, tag):
        t = consts.tile([P, w], F32, tag=tag)
        d = dram[:]
        src = bass.AP(tensor=d.tensor, offset=d.offset, ap=[[0, P], [1, w]])
        nc.gpsimd.dma_start(out=t, in_=src)
        return t

    qw_rep = rep_tile(QNW, HD, "qw_rep")
    kw_rep = rep_tile(KNW, HD, "kw_rep")

    # biases staged on partition 0 for K=1 matmul broadcast-adds
    qkvb_row = consts.tile([1, 3 * C], F32, tag="qkvb_row")
    if has_qkv_b:
        nc.sync.dma_start(out=qkvb_row,
                          in_=bass.AP(tensor=QKVB[:].tensor, offset=QKVB[:].offset,
                                      ap=[[0, 1], [1, 3 * C]]))
    pb_row = consts.tile([1, C], F32, tag="pb_row")
    if has_pj_b:
        nc.sync.dma_start(out=pb_row,
                          in_=bass.AP(tensor=PB[:].tensor, offset=PB[:].offset,
                                      ap=[[0, 1], [1, C]]))

    # rope table: cs[p, t, 0:64]=sin, [64:128]=cos for token t*128+p
    # (rope row t*128+p-1); token 0 -> sin=0, cos=1; pad tokens -> 0,0.
    cs = consts.tile([P, NTT, 2 * HD], F32)
    nc.vector.memset(cs[:, 8, :], 0.0)
    nc.gpsimd.dma_start(
        out=cs[1:128, 0, :],
        in_=bass.AP(tensor=ROPE[:].tensor, offset=ROPE[:].offset,
                    ap=[[2 * HD, 127], [1, 2 * HD]]))
    nc.gpsimd.dma_start(
        out=cs[:, 1:8, :],
        in_=bass.AP(tensor=ROPE[:].tensor, offset=ROPE[:].offset + 127 * 2 * HD,
                    ap=[[2 * HD, P], [P * 2 * HD, 7], [1, 2 * HD]]))
    nc.gpsimd.dma_start(
        out=cs[0:1, 8, :],
        in_=bass.AP(tensor=ROPE[:].tensor, offset=ROPE[:].offset + 1023 * 2 * HD,
                    ap=[[2 * HD, 1], [1, 2 * HD]]))
    nc.vector.memset(cs[0:1, 0, 0:HD], 0.0)
    nc.vector.memset(cs[0:1, 0, HD:2 * HD], 1.0)

    # rope-fused LN weights, replicated over 8 heads (full 512-wide tiles):
    #   wcX[p, t, h*64+d] = X_w[d] * cos[t,p,d]
    #   wsX[p, t, h*64+d] = X_w[d^pair] * sin[t,p,d]   (pair-swapped w)
    sin_v = cs[:, :, 0:HD].unsqueeze(2).broadcast_to([P, NTT, 8, HD])
    cos_v = cs[:, :, HD:2 * HD].unsqueeze(2).broadcast_to([P, NTT, 8, HD])

    # pair-swapped sin: sinsw[..., 2i] = sin[2i+1], sinsw[..., 2i+1] = sin[2i]
    sinsw = consts.tile([P, NTT, HD], F32, tag="sinsw")
    ssv = sinsw.rearrange("p t (i two) -> p t i two", two=2)
    sv = cs[:, :, 0:HD].rearrange("p t (i two) -> p t i two", two=2)
    nc.vector.tensor_copy(ssv[:, :, :, 0], sv[:, :, :, 1])
    nc.vector.tensor_copy(ssv[:, :, :, 1], sv[:, :, :, 0])
    sinsw_v = sinsw.unsqueeze(2).broadcast_to([P, NTT, 8, HD])

    def make_wcs(w_rep, tagp):
        # wc[d] = w[d]*cos[d]; ws[d] = w[d]*sin[d^1]  (so tmp2 = z*ws holds,
        # at slot 2i+1, the exact term subtracted by even output 2i and v.v.)
        # stored unreplicated [P, NTT, HD]; broadcast over heads at use site.
        wc = consts.tile([P, NTT, HD], F32, tag=tagp + "wc")
        ws = consts.tile([P, NTT, HD], F32, tag=tagp + "ws")
        wb = w_rep.unsqueeze(1).broadcast_to([P, NTT, HD])
        nc.vector.tensor_mul(wc, cs[:, :, HD:2 * HD], wb)
        nc.vector.tensor_mul(ws, sinsw, wb)
        return wc, ws

    wc_q, ws_q = make_wcs(qw_rep, "q")
    wc_k, ws_k = make_wcs(kw_rep, "k")

    # optional LN bias rope tables: bc[p,t,h*64+d] = b[d]*cos + swap(b)[d]*sin
    def make_bc(B_dram, tagp):
        b_rep = rep_tile(B_dram, HD, tagp + "b_rep")
        bsw = consts.tile([P, HD], F32, tag=tagp + "bsw")
        bswv = bsw.rearrange("p (i two) -> p i two", two=2)
        bv = b_rep.rearrange("p (i two) -> p i two", two=2)
        nc.vector.tensor_copy(bswv[:, :, 0], bv[:, :, 1])
        nc.vector.tensor_scalar_mul(bswv[:, :, 0], bswv[:, :, 0], -1.0)
        nc.vector.tensor_copy(bswv[:, :, 1], bv[:, :, 0])
        bc = consts.tile([P, NTT, 8, HD], F32, tag=tagp + "bc")
        bb = b_rep.unsqueeze(1).unsqueeze(1).broadcast_to([P, NTT, 8, HD])
        bswb = bsw.unsqueeze(1).unsqueeze(1).broadcast_to([P, NTT, 8, HD])
        nc.vector.tensor_mul(bc, cos_v, bb)
        bcs = consts.tile([P, NTT, 8, HD], F32, tag=tagp + "bcs")
        nc.vector.tensor_mul(bcs, sin_v, bswb)
        nc.vector.tensor_add(bc, bc, bcs)
        bcb = consts.tile([P, NTT, 8, HD], BF16, tag=tagp + "bcb")
        nc.vector.tensor_copy(bcb, bc)
        return bcb

    bc_q = make_bc(QNB, "qn") if has_qn_b else None
    bc_k = make_bc(KNB, "kn") if has_kn_b else None

    # persistent arenas
    xT = arena.tile([P, 8, NTOK], BF16, tag="xT")     # x^T [C, tok]
    qT = arena.tile([P, 8, NTOK], BF16, tag="qT")     # head-pair-major q^T
    kT = arena.tile([P, 8, NTOK], BF16, tag="kT")
    vA = arena.tile([P, NTT, H, HD + 1], BF16, tag="vA")
    oT = arena.tile([P, 8, NTOK], BF16, tag="oT")
    nc.gpsimd.memset(oT[:, :, 1024:NTOK], 0.0)

    # [v|1] aug: ones column everywhere, then zero ALL of pad tile 8 and
    # restore row 0 (token 1024). v-values written in phase B.
    nc.gpsimd.memset(vA[:, :, :, HD:HD + 1], 1.0)
    nc.gpsimd.memset(vA[:, 8, :, :], 0.0)
    nc.gpsimd.memset(vA[0:1, 8, :, HD:HD + 1], 1.0)

    if NPH < 1:
        return

    ph12 = ExitStack()
    ps_tp = ph12.enter_context(tc.tile_pool(name="ps_tp", bufs=3, space="PSUM"))

    # HAM clock pre-warm: ~5us of dependency-free PE work during the startup
    # DMA wait flips the clock gate to 8/8 (2.4 GHz) before real work lands.
    warm = ps_tp.tile([P, 4, P], BF16, tag="tp")
    for _ in range(12):
        for j in range(4):
            nc.tensor.transpose(warm[:, j, :], ident, ident)

    # ---------------- phase 1: x -> xT ----------------
    with tc.tile_pool(name="ph1", bufs=3) as ph1:
        for t in range(NTT):
            xt = ph1.tile([P, C], F32, tag="x_in")
            if t == 8:
                nc.vector.memset(xt, 0.0)
                nc.sync.dma_start(out=xt[0:1], in_=X[1024:1025, :])
            else:
                ways = 4 if t < 3 else 2
                step = P // ways
                for w in range(ways):
                    nc.sync.dma_start(
                        out=xt[w * step:(w + 1) * step],
                        in_=X[t * 128 + w * step:t * 128 + (w + 1) * step, :])
            xb = ph1.tile([P, C], BF16, tag="xb")
            nc.scalar.copy(xb, xt)
            for half in range(2):
                tp = ps_tp.tile([P, 4, P], BF16, tag="tp")
                for j in range(4):
                    cc = half * 4 + j
                    nc.tensor.transpose(tp[:, j, :], xb[:, cc * 128:(cc + 1) * 128],
                                        ident)
                nc.vector.tensor_copy(
                    xT[:, half * 4:half * 4 + 4, t * 128:(t + 1) * 128], tp)

    if NPH < 2:
        return
    # ---------------- phase 2: qkv + LN + rope ----------------
    NCH_ORDER = [0, 2, 4, 1, 3, 5]
    with tc.tile_pool(name="wld", bufs=3) as wld, \
         tc.tile_pool(name="wtp", bufs=2) as wtp, \
         tc.tile_pool(name="stg", bufs=4) as stg, \
         tc.tile_pool(name="sml", bufs=5) as sml, \
         tc.tile_pool(name="ps_pq", bufs=4, space="PSUM") as ps_pq:
        def emit_wblock(nch):
            wt = wtp.tile([P, 8, 512], BF16, tag="wt")
            for j4 in range(4):
                wl = wld.tile([P, C], F32, tag="wl")
                r0 = nch * 512 + j4 * 128
                nc.sync.dma_start(out=wl[0:64], in_=QKVW[r0:r0 + 64, :])
                nc.sync.dma_start(out=wl[64:128], in_=QKVW[r0 + 64:r0 + 128, :])
                wlb = wld.tile([P, C], BF16, tag="wlb")
                nc.scalar.copy(wlb, wl)
                for half in range(2):
                    tp = ps_tp.tile([P, 4, P], BF16, tag="tp")
                    for j in range(4):
                        cc = half * 4 + j
                        nc.tensor.transpose(tp[:, j, :],
                                            wlb[:, cc * 128:(cc + 1) * 128], ident)
                    nc.vector.tensor_copy(
                        wt[:, half * 4:half * 4 + 4, j4 * 128:(j4 + 1) * 128], tp)
            return wt

        def emit_chunk(nch, t, wt):
            is_q = nch < 2
            hs = (nch & 1) * 8
            if True:
                pq = ps_pq.tile([P, 512], F32, tag="pq")
                for cc in range(8):
                    nc.tensor.matmul(pq, xT[:, cc, t * 128:(t + 1) * 128],
                                     wt[:, cc, :],
                                     start=(cc == 0), stop=(cc == 7 and not has_qkv_b))
                if has_qkv_b:
                    nc.tensor.matmul(pq, _r(ones_row),
                                     _r(qkvb_row[:, nch * 512:(nch + 1) * 512]),
                                     start=False, stop=True)
                pqv = pq.rearrange("p (h d) -> p h d", d=HD)
                if nch >= 4:
                    # v chunk: psum -> vA (pad rows of tile 8 stay zero)
                    rw = 1 if t == 8 else P
                    nc.scalar.copy(vA[:rw, t, hs:hs + 8, 0:HD], pqv[:rw])
                    return
                # --- LN stats (per token-row, per head) ---
                s1 = sml.tile([P, 8], F32, tag="s1")
                nc.vector.tensor_reduce(s1, pqv, axis=AXM.X, op=ALU.add)
                sq = stg.tile([P, 512], F32, tag="sq")
                nc.scalar.square(sq, pq)
                s2 = sml.tile([P, 8], F32, tag="s2")
                nc.vector.tensor_reduce(s2, sq.rearrange("p (h d) -> p h d", d=HD),
                                        axis=AXM.X, op=ALU.add)
                mu = sml.tile([P, 8], F32, tag="mu")
                nc.vector.tensor_scalar_mul(mu, s1, 1.0 / HD)
                mus = sml.tile([P, 8], F32, tag="mus")
                nc.vector.tensor_mul(mus, mu, mu)
                var = sml.tile([P, 8], F32, tag="var")
                nc.vector.scalar_tensor_tensor(var, s2, 1.0 / HD, mus,
                                               op0=ALU.mult, op1=ALU.subtract)
                sd = sml.tile([P, 8], F32, tag="sd")
                nc.scalar.activation(sd, var, AF.Sqrt, bias=eps_t)
                rstd = sml.tile([P, 8], F32, tag="rstd")
                nc.vector.reciprocal_approx_fast(rstd, sd)
                # --- normalize ---
                mu_b = mu.unsqueeze(2).broadcast_to([P, 8, HD])
                rstd_b = rstd.unsqueeze(2).broadcast_to([P, 8, HD])
                t1 = stg.tile([P, 8, HD], F32, tag="t1")
                nc.vector.tensor_sub(t1, pqv, mu_b)
                z = stg.tile([P, 8, HD], F32, tag="z")
                nc.vector.tensor_mul(z, t1, rstd_b)
                # --- rope: out_e = z*wc_e - z_o*ws_e ; out_o = z*wc_o + z_e*ws_o
                wc, ws = (wc_q, ws_q) if is_q else (wc_k, ws_k)
                wc_t = wc[:, t].unsqueeze(1).broadcast_to([P, 8, HD])
                ws_t = ws[:, t].unsqueeze(1).broadcast_to([P, 8, HD])
                tmp1 = stg.tile([P, 8, HD], F32, tag="tmp1")
                nc.vector.tensor_mul(tmp1, z, wc_t)
                tmp2 = stg.tile([P, 8, HD], F32, tag="tmp2")
                nc.vector.tensor_mul(tmp2, z, ws_t)
                qro = stg.tile([P, 512], BF16, tag="qro")
                qrov = qro.rearrange("p (h i two) -> p h i two", h=8, two=2)
                t1v = tmp1.rearrange("p h (i two) -> p h i two", two=2)
                t2v = tmp2.rearrange("p h (i two) -> p h i two", two=2)
                nc.vector.tensor_sub(qrov[:, :, :, 0], t1v[:, :, :, 0],
                                     t2v[:, :, :, 1])
                nc.gpsimd.tensor_add(qrov[:, :, :, 1], t1v[:, :, :, 1],
                                     t2v[:, :, :, 0])
                bc = bc_q if is_q else bc_k
                if bc is not None:
                    nc.vector.tensor_add(qro, qro,
                                         bc[:, t].rearrange("p h d -> p (h d)"))
                # --- transpose head-pairs -> qT/kT ---
                dstT = qT if is_q else kT
                tp = ps_tp.tile([P, 4, P], BF16, tag="tp")
                for hp in range(4):
                    nc.tensor.transpose(tp[:, hp, :], qro[:, hp * 128:(hp + 1) * 128],
                                        ident)
                if os.environ.get("KSPLITCP"):
                    for hp in range(4):
                        nc.vector.tensor_copy(
                            dstT[:, hs // 2 + hp, t * 128:(t + 1) * 128],
                            tp[:, hp, :])
                else:
                    nc.vector.tensor_copy(
                        dstT[:, hs // 2:hs // 2 + 4, t * 128:(t + 1) * 128], tp)

        # interleave nch pairs so each LN chain's latency hides behind the
        # other stream's matmuls on the PE
        for na, nb in [(0, 2), (4, 1), (3, 5)]:
            wta = emit_wblock(na)
            wtb = emit_wblock(nb)
            for t in range(NTT):
                emit_chunk(na, t, wta)
                emit_chunk(nb, t, wtb)

        # ---------------- proj_w -> projT (tail of phase 2) ----------------
        projT = arena.tile([P, 8, C], BF16, tag="xT")
        for j in ([] if os.environ.get("KSUBX") else range(8)):
            wl2 = wld.tile([P, C], F32, tag="wl")
            nc.sync.dma_start(out=wl2, in_=PW[j * 128:(j + 1) * 128, :])
            wl2b = wld.tile([P, C], BF16, tag="wlb")
            nc.scalar.copy(wl2b, wl2)
            for half in range(2):
                tp = ps_tp.tile([P, 4, P], BF16, tag="tp")
                for jj in range(4):
                    cc = half * 4 + jj
                    nc.tensor.transpose(tp[:, jj, :],
                                        wl2b[:, cc * 128:(cc + 1) * 128], ident)
                nc.vector.tensor_copy(
                    projT[:, half * 4:half * 4 + 4, j * 128:(j + 1) * 128], tp)

    ph12.close()

    if NPH < 4:
        return

    if os.environ.get("KSDPA") == "base":
        # baseline-style SDPA (known-good on HW)
        QCHB = [(0, 512), (512, 512), (897, 128)]
        ones_col_r = ones_col
        with tc.tile_pool(name="expp", bufs=10) as expp, \
             tc.tile_pool(name="sml", bufs=4) as sml, \
             tc.tile_pool(name="psq", bufs=4, space="PSUM") as psq, \
             tc.tile_pool(name="pso", bufs=2, space="PSUM") as psop, \
             tc.tile_pool(name="psb", bufs=2, space="PSUM") as psbp:
            for h in range(H):
                g, half = h // 2, (h % 2) * 64
                ex_tiles = []
                for kt in range(NTT):
                    rw = 128 if kt < 8 else 1
                    ex = expp.tile([P, NT], BF16, tag="expS")
                    ex_tiles.append(ex)
                    for (q0, qn) in QCHB:
                        pss = psq.tile([P, 512], F32, tag="ps")
                        nc.tensor.matmul(
                            pss[:rw, :qn],
                            kT[half:half + 64, g, kt * 128:kt * 128 + rw],
                            qT[half:half + 64, g, q0:q0 + qn],
                            start=True, stop=True)
                        nc.scalar.activation(ex[:rw, q0:q0 + qn], pss[:rw, :qn],
                                             AF.Exp, scale=SCALE)
                for (q0, qn) in QCHB:
                    po = psop.tile([HD + 1, 512], F32, tag="po")
                    for kt in range(NTT):
                        rw = 128 if kt < 8 else 1
                        nc.tensor.matmul(po[:, :qn], vA[:rw, kt, h, :],
                                         ex_tiles[kt][:rw, q0:q0 + qn],
                                         start=(kt == 0), stop=(kt == 8))
                    rs = sml.tile([1, 512], F32, tag="rs")
                    if os.environ.get("KRECIP") == "approx":
                        rs0 = sml.tile([1, 512], F32, tag="rs0")
                        nc.vector.reciprocal_approx_fast(rs0[:, :qn],
                                                         po[HD:HD + 1, :qn])
                        with nc.allow_low_precision(reason="softmax denom"):
                            nc.vector.tensor_copy(_r(rs[:, :qn]), rs0[:, :qn])
                    else:
                        with nc.allow_low_precision(reason="softmax denom"):
                            nc.vector.reciprocal(_r(rs[:, :qn]), po[HD:HD + 1, :qn])
                    pb = psbp.tile([HD, 512], F32, tag="pb")
                    nc.tensor.matmul(pb[:, :qn], _r(ones_col), _r(rs[:, :qn]),
                                     start=True, stop=True)
                    nc.vector.tensor_copy(oT[half:half + 64, g, q0:q0 + qn],
                                          po[:HD, :qn])
                    nc.vector.tensor_mul(oT[half:half + 64, g, q0:q0 + qn],
                                         oT[half:half + 64, g, q0:q0 + qn],
                                         pb[:, :qn])
        _run_phase5 = True
    else:
        _run_phase5 = False

    if _run_phase5:
        pass
    elif True:
        pass
    # ---------------- phase 4: SDPA ----------------
    if os.environ.get("KSDPA") == "base":
        pass
    else:
     with tc.tile_pool(name="expp", bufs=20) as expp, \
         tc.tile_pool(name="rsp", bufs=3) as rsp, \
         tc.tile_pool(name="rsbp", bufs=1) as rsbp, \
         tc.tile_pool(name="ps_qs", bufs=2, space="PSUM") as ps_qs, \
         tc.tile_pool(name="ps_po", bufs=2, space="PSUM") as ps_po:
        ex_store = {}

        def emit_qk(h):
            g, half = h // 2, (h % 2) * 64
            ex_store[h] = []
            for kt in range(NTT):
                qs = ps_qs.tile([P, NTOK], F32, tag="qs")
                lsrc, rsrc = (kT, qT) if not os.environ.get("KSUBX") else (xT, xT)
                for (q0, qn, pc) in QCH:
                    nc.tensor.matmul(
                        qs[:, pc:pc + qn],
                        lsrc[half:half + 64, g, kt * 128:(kt + 1) * 128],
                        rsrc[half:half + 64, g, q0:q0 + qn],
                        start=True, stop=True)
                ex = expp.tile([P, NTOK], BF16, tag="ex")
                nc.scalar.activation(ex, qs, AF.Exp, scale=SCALE)
                ex_store[h].append(ex)
                if os.environ.get("KDBG2") and h == 0 and kt == 0:
                    dbga = arena.tile([P, 1024], F32, tag="dbga")
                    nc.vector.tensor_copy(dbga, ex[:, 0:1024])
                    nc.sync.dma_start(out=OUT[0:128, :], in_=dbga)
                    dbgs = arena.tile([P, 1024], F32, tag="dbgs")
                    nc.vector.tensor_copy(dbgs, qs[:, 0:1024])
                    nc.sync.dma_start(out=OUT[640:768, :], in_=dbgs)

        def emit_pv(h):
            # fast drain: copy v-part -> oT (unnormalized) and sums row ->
            # sums_all[h]; normalization deferred to emit_norm.
            g, half = h // 2, (h % 2) * 64
            exs = ex_store.pop(h)
            pos = []
            for ci, (q0, qn, pc) in enumerate(QCH):
                po = ps_po.tile([P, 512], F32, tag="po")
                for kt in range(NTT):
                    nc.tensor.matmul(po[0:HD + 1, 0:qn], vA[:, kt, h, :],
                                     exs[kt][:, pc:pc + qn],
                                     start=(kt == 0), stop=(kt == 8))
                pos.append(po)
                if ci == 0:
                    continue
                _drain_chunk(h, ci - 1, pos[ci - 1])
            _drain_chunk(h, 2, pos[2])

        sums_store = {}

        def _drain_chunk(h, ci, po):
            g, half = h // 2, (h % 2) * 64
            q0, qn, pc = QCH[ci]
            nc.vector.tensor_copy(oT[half:half + 64, g, q0:q0 + qn],
                                  po[0:HD, 0:qn])
            if ci == 0:
                sums = rsp.tile([1, NTOK], F32, tag="sums")
                nc.vector.memset(sums[:, 1024:NTOK], 1.0)
                sums_store[h] = sums
            nc.vector.tensor_copy(sums_store[h][:, q0:q0 + qn],
                                  po[HD:HD + 1, 0:qn])

        rsb_store = {}

        def emit_norm_a(h):
            # recip + partition-broadcast (DVE then GpSimd)
            sums = sums_store.pop(h)
            rs = rsp.tile([1, NTOK], F32, tag="rs")
            nc.vector.reciprocal_approx_fast(rs, sums)
            rsb = rsbp.tile([P, NTOK], F32, tag="rsb")
            nc.gpsimd.partition_broadcast(rsb, rs, channels=P)
            rsb_store[h] = rsb

        def emit_norm_b(h):
            # in-place mult, emitted a head later so the DVE queue never
            # head-blocks waiting on the GpSimd broadcast
            g, half = h // 2, (h % 2) * 64
            rsb = rsb_store.pop(h)
            dst = oT[half:half + 64, g, :]
            nc.vector.tensor_mul(dst, rsb[half:half + HD, :], dst)

        emit_qk(0)
        for h in range(H):
            if h + 1 < H:
                emit_qk(h + 1)
            emit_pv(h)
            if h >= 1:
                emit_norm_a(h - 1)
            if h >= 2:
                emit_norm_b(h - 2)
        emit_norm_a(H - 1)
        emit_norm_b(H - 2)
        emit_norm_b(H - 1)

    if os.environ.get("KDBG"):
        dbg = arena.tile([P, 1024], F32, tag="dbg")
        nc.vector.tensor_copy(dbg, qT[:, 0, 0:1024])
        nc.sync.dma_start(out=OUT[0:128, :], in_=dbg)
        dbg2 = arena.tile([P, 1024], F32, tag="dbg2")
        nc.vector.tensor_copy(dbg2, kT[:, 0, 0:1024])
        nc.sync.dma_start(out=OUT[128:256, :], in_=dbg2)
        dbg3 = arena.tile([P, 1024], F32, tag="dbg3")
        nc.vector.tensor_copy(dbg3[:, 0:975],
                              vA[:, 0, 0:15, :].rearrange("p h d -> p (h d)"))
        nc.sync.dma_start(out=OUT[256:384, :], in_=dbg3)
        dbg4 = arena.tile([P, 1024], F32, tag="dbg4")
        nc.vector.tensor_copy(dbg4, oT[:, 0, 0:1024])
        nc.sync.dma_start(out=OUT[384:512, :], in_=dbg4)
        dbg5 = arena.tile([P, 1024], F32, tag="dbg5")
        nc.vector.tensor_copy(dbg5, projT[:, 0, 0:1024])
        nc.sync.dma_start(out=OUT[512:640, :], in_=dbg5)
        return

    if NPH < 5:
        return
    # ---------------- phase 5: proj ----------------
    with tc.tile_pool(name="ps_py", bufs=4, space="PSUM") as ps_py, \
         tc.tile_pool(name="yp", bufs=4) as yp:
        for t in range(NTT):
            rw = 1 if t == 8 else P
            for n2 in range(2):
                py = ps_py.tile([P, 512], F32, tag="py")
                for cc in range(8):
                    nc.tensor.matmul(py[:rw], oT[:, cc, t * 128:t * 128 + rw],
                                     projT[:, cc, n2 * 512:(n2 + 1) * 512],
                                     start=(cc == 0), stop=(cc == 7 and not has_pj_b))
                if has_pj_b:
                    nc.tensor.matmul(py[:rw], _r(ones_row[:, :rw]),
                                     _r(pb_row[:, n2 * 512:(n2 + 1) * 512]),
                                     start=False, stop=True)
                ysb = yp.tile([P, 512], F32, tag="ysb")
                nc.vector.tensor_copy(ysb[:rw], py[:rw])
                nc.sync.dma_start(
                    out=OUT[t * 128:t * 128 + rw, n2 * 512:(n2 + 1) * 512],
                    in_=ysb[:rw])


_NC_CACHE = {}


def _build_nc(flags):
    if flags in _NC_CACHE:
        return _NC_CACHE[flags]
    nc = _bacc.Bacc()
    X = nc.declare_dram_parameter("x", [NT, C], F32, isOutput=False)
    ROPE = nc.declare_dram_parameter("rope", [NT - 1, 2 * HD], F32, isOutput=False)
    QKVW = nc.declare_dram_parameter("qkv_w", [3 * C, C], F32, isOutput=False)
    QKVB = nc.declare_dram_parameter("qkv_b", [3 * C], F32, isOutput=False)
    QNW = nc.declare_dram_parameter("qn_w", [HD], F32, isOutput=False)
    QNB = nc.declare_dram_parameter("qn_b", [HD], F32, isOutput=False)
    KNW = nc.declare_dram_parameter("kn_w", [HD], F32, isOutput=False)
    KNB = nc.declare_dram_parameter("kn_b", [HD], F32, isOutput=False)
    PW = nc.declare_dram_parameter("proj_w", [C, C], F32, isOutput=False)
    PB = nc.declare_dram_parameter("proj_b", [C], F32, isOutput=False)
    OUT = nc.declare_dram_parameter("out", [NT, C], F32, isOutput=True)
    with ExitStack() as ctx:
        tc = ctx.enter_context(tile.TileContext(nc))
        build_kernel(ctx, tc, X, ROPE, QKVW, QKVB, QNW, QNB, KNW, KNB, PW, PB,
                     OUT, *flags)
    nc.finalize()
    _NC_CACHE[flags] = nc
    return nc


def kernel(x, rope, qkv_w, qkv_b, qn_w, qn_b, kn_w, kn_b, proj_w, proj_b):
    global LAST_RESULT
    flags = (bool(np.any(qkv_b)), bool(np.any(qn_b)), bool(np.any(kn_b)),
             bool(np.any(proj_b)))
    nc = _build_nc(flags)
    shared = dict(rope=np.asarray(rope, np.float32),
                  qkv_w=np.asarray(qkv_w, np.float32),
                  qkv_b=np.asarray(qkv_b, np.float32),
                  qn_w=np.asarray(qn_w, np.float32),
                  qn_b=np.asarray(qn_b, np.float32),
                  kn_w=np.asarray(kn_w, np.float32),
                  kn_b=np.asarray(kn_b, np.float32),
                  proj_w=np.asarray(proj_w, np.float32),
                  proj_b=np.asarray(proj_b, np.float32))
    x = np.asarray(x, np.float32)
    in_maps = [dict(x=np.ascontiguousarray(x[i]), **shared) for i in range(B)]
    res = run_bass_kernel_spmd(nc, in_maps, list(range(B)))
    LAST_RESULT = res
    return np.stack([res.results[i]["out"] for i in range(B)], axis=0)


# revision 65
# speedup vs baseline: 1.0141x; 1.0141x over previous
"""AttentionRope TRN2 kernel: data-parallel over batch (1 batch elem / core).

v2 redesign vs baseline:
  - tokens padded 1025 -> 1152 (9 full 128-tiles); pad k-tokens neutralized
    by zeroing their [v|1] rows (no numerator/denominator contribution),
    pad q-tokens never stored.
  - QK for each (head, k-tile) lands in one 3-bank [128,1152] PSUM tile ->
    ONE exp activation per k-tile (scale folded).
  - PV accumulates [v|1]^T @ expS into psum rows 0..64; softmax denominator
    reciprocal via reciprocal_approx_fast on row 64; ones-outer-product
    (f32r matmul, tile_position col=64) broadcasts recip into rows 64..127
    of the SAME psum bank; one fused DVE mult writes oT.
  - LN+rope: batched stats via X-axis reduces + broadcast-view normalize;
    rope via premultiplied (w*cos, swap(w)*sin) tables and two strided
    even/odd combines (no separate rotate pass). Work split DVE/GpSimd/ACT.
  - biases (all zero in practice) added via K=1 f32r matmuls into psum.
  - software pipelining: QK(h+1) emitted before PV(h); proj output DMA'd
    from PSUM.
"""

import os
import numpy as np
from contextlib import ExitStack

import concourse.bass as bass
from concourse import bacc as _bacc
import concourse.mybir as mybir
import concourse.tile as tile
from concourse.bass_utils import run_bass_kernel_spmd
from concourse.masks import make_identity

B, NT, C = 8, 1025, 1024
H, HD = 16, 64
EPS = 1e-6
SCALE = HD ** -0.5
P = 128
NTT = 9                 # token tiles
NTOK = NTT * P          # padded token count 1152
F32 = mybir.dt.float32
F32R = mybir.dt.float32r
BF16 = mybir.dt.bfloat16
AF = mybir.ActivationFunctionType
ALU = mybir.AluOpType
AXM = mybir.AxisListType

LAST_RESULT = None

# q chunks: (q0, qn, psum col offset). chunk 3 overlaps tokens 897..1024 so
# every computed q column is a real token.
QCH = [(0, 512, 0), (512, 512, 512), (897, 128, 1024)]


def _r(ap):
    return ap.bitcast(F32R)


def build_kernel(ctx, tc, X, ROPE, QKVW, QKVB, QNW, QNB, KNW, KNB, PW, PB, OUT,
                 has_qkv_b, has_qn_b, has_kn_b, has_pj_b):
    NPH = int(os.environ.get("KPHASES", "9"))
    nc = tc.nc

    consts = ctx.enter_context(tc.tile_pool(name="consts", bufs=1))
    arena = ctx.enter_context(tc.tile_pool(name="arena", bufs=1))

    ident0 = consts.tile([P, P], F32, tag="ident0")
    make_identity(nc, ident0)
    ident = consts.tile([P, P], BF16, tag="ident")
    nc.vector.tensor_copy(ident, ident0)
    ones64 = consts.tile([1, HD], F32, tag="ones64")
    nc.vector.memset(ones64, 1.0)
    ones_col = consts.tile([1, HD], F32, tag="ones_col")
    nc.vector.tensor_copy(_r(ones_col), ones64)
    ones_row = consts.tile([1, P], F32, tag="ones_row")
    nc.vector.memset(ones_row, 1.0)
    ones_row_r = consts.tile([1, P], F32, tag="ones_row_r")
    nc.vector.tensor_copy(_r(ones_row_r), ones_row)
    eps_t = consts.tile([P, 1], F32)
    nc.vector.memset(eps_t, EPS)

    def rep_tile(dram, w, tag):
        t = consts.tile([P, w], F32, tag=tag)
        d = dram[:]
        src = bass.AP(tensor=d.tensor, offset=d.offset, ap=[[0, P], [1, w]])
        nc.gpsimd.dma_start(out=t, in_=src)
        return t

    qw_rep = rep_tile(QNW, HD, "qw_rep")
    kw_rep = rep_tile(KNW, HD, "kw_rep")

    # biases staged on partition 0 for K=1 matmul broadcast-adds
    qkvb_row = consts.tile([1, 3 * C], F32, tag="qkvb_row")
    if has_qkv_b:
        nc.sync.dma_start(out=qkvb_row,
                          in_=bass.AP(tensor=QKVB[:].tensor, offset=QKVB[:].offset,
                                      ap=[[0, 1], [1, 3 * C]]))
    pb_row = consts.tile([1, C], F32, tag="pb_row")
    if has_pj_b:
        nc.sync.dma_start(out=pb_row,
                          in_=bass.AP(tensor=PB[:].tensor, offset=PB[:].offset,
                                      ap=[[0, 1], [1, C]]))

    # rope table: cs[p, t, 0:64]=sin, [64:128]=cos for token t*128+p
    # (rope row t*128+p-1); token 0 -> sin=0, cos=1; pad tokens -> 0,0.
    cs = consts.tile([P, NTT, 2 * HD], F32)
    nc.vector.memset(cs[:, 8, :], 0.0)
    nc.gpsimd.dma_start(
        out=cs[1:128, 0, :],
        in_=bass.AP(tensor=ROPE[:].tensor, offset=ROPE[:].offset,
                    ap=[[2 * HD, 127], [1, 2 * HD]]))
    nc.gpsimd.dma_start(
        out=cs[:, 1:8, :],
        in_=bass.AP(tensor=ROPE[:].tensor, offset=ROPE[:].offset + 127 * 2 * HD,
                    ap=[[2 * HD, P], [P * 2 * HD, 7], [1, 2 * HD]]))
    nc.gpsimd.dma_start(
        out=cs[0:1, 8, :],
        in_=bass.AP(tensor=ROPE[:].tensor, offset=ROPE[:].offset + 1023 * 2 * HD,
                    ap=[[2 * HD, 1], [1, 2 * HD]]))
    nc.vector.memset(cs[0:1, 0, 0:HD], 0.0)
    nc.vector.memset(cs[0:1, 0, HD:2 * HD], 1.0)

    # rope-fused LN weights, replicated over 8 heads (full 512-wide tiles):
    #   wcX[p, t, h*64+d] = X_w[d] * cos[t,p,d]
    #   wsX[p, t, h*64+d] = X_w[d^pair] * sin[t,p,d]   (pair-swapped w)
    sin_v = cs[:, :, 0:HD].unsqueeze(2).broadcast_to([P, NTT, 8, HD])
    cos_v = cs[:, :, HD:2 * HD].unsqueeze(2).broadcast_to([P, NTT, 8, HD])

    # pair-swapped sin: sinsw[..., 2i] = sin[2i+1], sinsw[..., 2i+1] = sin[2i]
    sinsw = consts.tile([P, NTT, HD], F32, tag="sinsw")
    ssv = sinsw.rearrange("p t (i two) -> p t i two", two=2)
    sv = cs[:, :, 0:HD].rearrange("p t (i two) -> p t i two", two=2)
    nc.vector.tensor_copy(ssv[:, :, :, 0], sv[:, :, :, 1])
    nc.vector.tensor_copy(ssv[:, :, :, 1], sv[:, :, :, 0])
    sinsw_v = sinsw.unsqueeze(2).broadcast_to([P, NTT, 8, HD])

    def make_wcs(w_rep, tagp):
        # wc[d] = w[d]*cos[d]; ws[d] = w[d]*sin[d^1]  (so tmp2 = z*ws holds,
        # at slot 2i+1, the exact term subtracted by even output 2i and v.v.)
        # stored unreplicated [P, NTT, HD]; broadcast over heads at use site.
        wc = consts.tile([P, NTT, HD], F32, tag=tagp + "wc")
        ws = consts.tile([P, NTT, HD], F32, tag=tagp + "ws")
        wb = w_rep.unsqueeze(1).broadcast_to([P, NTT, HD])
        nc.vector.tensor_mul(wc, cs[:, :, HD:2 * HD], wb)
        nc.vector.tensor_mul(ws, sinsw, wb)
        return wc, ws

    wc_q, ws_q = make_wcs(qw_rep, "q")
    wc_k, ws_k = make_wcs(kw_rep, "k")

    # optional LN bias rope tables: bc[p,t,h*64+d] = b[d]*cos + swap(b)[d]*sin
    def make_bc(B_dram, tagp):
        b_rep = rep_tile(B_dram, HD, tagp + "b_rep")
        bsw = consts.tile([P, HD], F32, tag=tagp + "bsw")
        bswv = bsw.rearrange("p (i two) -> p i two", two=2)
        bv = b_rep.rearrange("p (i two) -> p i two", two=2)
        nc.vector.tensor_copy(bswv[:, :, 0], bv[:, :, 1])
        nc.vector.tensor_scalar_mul(bswv[:, :, 0], bswv[:, :, 0], -1.0)
        nc.vector.tensor_copy(bswv[:, :, 1], bv[:, :, 0])
        bc = consts.tile([P, NTT, 8, HD], F32, tag=tagp + "bc")
        bb = b_rep.unsqueeze(1).unsqueeze(1).broadcast_to([P, NTT, 8, HD])
        bswb = bsw.unsqueeze(1).unsqueeze(1).broadcast_to([P, NTT, 8, HD])
        nc.vector.tensor_mul(bc, cos_v, bb)
        bcs = consts.tile([P, NTT, 8, HD], F32, tag=tagp + "bcs")
        nc.vector.tensor_mul(bcs, sin_v, bswb)
        nc.vector.tensor_add(bc, bc, bcs)
        bcb = consts.tile([P, NTT, 8, HD], BF16, tag=tagp + "bcb")
        nc.vector.tensor_copy(bcb, bc)
        return bcb

    bc_q = make_bc(QNB, "qn") if has_qn_b else None
    bc_k = make_bc(KNB, "kn") if has_kn_b else None

    # persistent arenas
    xT = arena.tile([P, 8, NTOK], BF16, tag="xT")     # x^T [C, tok]
    qT = arena.tile([P, 8, NTOK], BF16, tag="qT")     # head-pair-major q^T
    kT = arena.tile([P, 8, NTOK], BF16, tag="kT")
    vA = arena.tile([P, NTT, H, HD + 1], BF16, tag="vA")
    oT = arena.tile([P, 8, NTOK], BF16, tag="oT")
    nc.gpsimd.memset(oT[:, :, 1024:NTOK], 0.0)

    # [v|1] aug: ones column everywhere, then zero ALL of pad tile 8 and
    # restore row 0 (token 1024). v-values written in phase B.
    nc.gpsimd.memset(vA[:, :, :, HD:HD + 1], 1.0)
    nc.gpsimd.memset(vA[:, 8, :, :], 0.0)
    nc.gpsimd.memset(vA[0:1, 8, :, HD:HD + 1], 1.0)

    if NPH < 1:
        return

    ph12 = ExitStack()
    ps_tp = ph12.enter_context(tc.tile_pool(name="ps_tp", bufs=3, space="PSUM"))

    # HAM clock pre-warm: ~5us of dependency-free PE work during the startup
    # DMA wait flips the clock gate to 8/8 (2.4 GHz) before real work lands.
    warm = ps_tp.tile([P, 4, P], BF16, tag="tp")
    for _ in range(12):
        for j in range(4):
            nc.tensor.transpose(warm[:, j, :], ident, ident)

    # ---------------- phase 1: x -> xT ----------------
    with tc.tile_pool(name="ph1", bufs=3) as ph1:
        for t in range(NTT):
            xt = ph1.tile([P, C], F32, tag="x_in")
            if t == 8:
                nc.vector.memset(xt, 0.0)
                nc.sync.dma_start(out=xt[0:1], in_=X[1024:1025, :])
            else:
                ways = 4 if t < 3 else 2
                step = P // ways
                for w in range(ways):
                    nc.sync.dma_start(
                        out=xt[w * step:(w + 1) * step],
                        in_=X[t * 128 + w * step:t * 128 + (w + 1) * step, :])
            xb = ph1.tile([P, C], BF16, tag="xb")
            nc.scalar.copy(xb, xt)
            for half in range(2):
                tp = ps_tp.tile([P, 4, P], BF16, tag="tp")
                for j in range(4):
                    cc = half * 4 + j
                    nc.tensor.transpose(tp[:, j, :], xb[:, cc * 128:(cc + 1) * 128],
                                        ident)
                nc.vector.tensor_copy(
                    xT[:, half * 4:half * 4 + 4, t * 128:(t + 1) * 128], tp)

    if NPH < 2:
        return
    # ---------------- phase 2: qkv + LN + rope ----------------
    NCH_ORDER = [0, 2, 4, 1, 3, 5]
    with tc.tile_pool(name="wld", bufs=3) as wld, \
         tc.tile_pool(name="wtp", bufs=2) as wtp, \
         tc.tile_pool(name="stg", bufs=4) as stg, \
         tc.tile_pool(name="sml", bufs=5) as sml, \
         tc.tile_pool(name="ps_pq", bufs=4, space="PSUM") as ps_pq:
        def emit_wblock(nch):
            wt = wtp.tile([P, 8, 512], BF16, tag="wt")
            for j4 in range(4):
                wl = wld.tile([P, C], F32, tag="wl")
                r0 = nch * 512 + j4 * 128
                nc.sync.dma_start(out=wl[0:64], in_=QKVW[r0:r0 + 64, :])
                nc.sync.dma_start(out=wl[64:128], in_=QKVW[r0 + 64:r0 + 128, :])
                wlb = wld.tile([P, C], BF16, tag="wlb")
                nc.scalar.copy(wlb, wl)
                for half in range(2):
                    tp = ps_tp.tile([P, 4, P], BF16, tag="tp")
                    for j in range(4):
                        cc = half * 4 + j
                        nc.tensor.transpose(tp[:, j, :],
                                            wlb[:, cc * 128:(cc + 1) * 128], ident)
                    nc.vector.tensor_copy(
                        wt[:, half * 4:half * 4 + 4, j4 * 128:(j4 + 1) * 128], tp)
            return wt

        def emit_chunk(nch, t, wt):
            is_q = nch < 2
            hs = (nch & 1) * 8
            if True:
                pq = ps_pq.tile([P, 512], F32, tag="pq")
                for cc in range(8):
                    nc.tensor.matmul(pq, xT[:, cc, t * 128:(t + 1) * 128],
                                     wt[:, cc, :],
                                     start=(cc == 0), stop=(cc == 7 and not has_qkv_b))
                if has_qkv_b:
                    nc.tensor.matmul(pq, _r(ones_row),
                                     _r(qkvb_row[:, nch * 512:(nch + 1) * 512]),
                                     start=False, stop=True)
                pqv = pq.rearrange("p (h d) -> p h d", d=HD)
                if nch >= 4:
                    # v chunk: psum -> vA (pad rows of tile 8 stay zero)
                    rw = 1 if t == 8 else P
                    nc.scalar.copy(vA[:rw, t, hs:hs + 8, 0:HD], pqv[:rw])
                    return
                # --- LN stats (per token-row, per head) ---
                s1 = sml.tile([P, 8], F32, tag="s1")
                nc.vector.tensor_reduce(s1, pqv, axis=AXM.X, op=ALU.add)
                sq = stg.tile([P, 512], F32, tag="sq")
                nc.scalar.square(sq, pq)
                s2 = sml.tile([P, 8], F32, tag="s2")
                nc.vector.tensor_reduce(s2, sq.rearrange("p (h d) -> p h d", d=HD),
                                        axis=AXM.X, op=ALU.add)
                mu = sml.tile([P, 8], F32, tag="mu")
                nc.vector.tensor_scalar_mul(mu, s1, 1.0 / HD)
                mus = sml.tile([P, 8], F32, tag="mus")
                nc.vector.tensor_mul(mus, mu, mu)
                var = sml.tile([P, 8], F32, tag="var")
                nc.vector.scalar_tensor_tensor(var, s2, 1.0 / HD, mus,
                                               op0=ALU.mult, op1=ALU.subtract)
                sd = sml.tile([P, 8], F32, tag="sd")
                nc.scalar.activation(sd, var, AF.Sqrt, bias=eps_t)
                rstd = sml.tile([P, 8], F32, tag="rstd")
                nc.vector.reciprocal_approx_fast(rstd, sd)
                # --- normalize ---
                mu_b = mu.unsqueeze(2).broadcast_to([P, 8, HD])
                rstd_b = rstd.unsqueeze(2).broadcast_to([P, 8, HD])
                t1 = stg.tile([P, 8, HD], F32, tag="t1")
                nc.vector.tensor_sub(t1, pqv, mu_b)
                z = stg.tile([P, 8, HD], F32, tag="z")
                nc.vector.tensor_mul(z, t1, rstd_b)
                # --- rope: out_e = z*wc_e - z_o*ws_e ; out_o = z*wc_o + z_e*ws_o
                wc, ws = (wc_q, ws_q) if is_q else (wc_k, ws_k)
                wc_t = wc[:, t].unsqueeze(1).broadcast_to([P, 8, HD])
                ws_t = ws[:, t].unsqueeze(1).broadcast_to([P, 8, HD])
                tmp1 = stg.tile([P, 8, HD], F32, tag="tmp1")
                nc.vector.tensor_mul(tmp1, z, wc_t)
                tmp2 = stg.tile([P, 8, HD], F32, tag="tmp2")
                nc.vector.tensor_mul(tmp2, z, ws_t)
                qro = stg.tile([P, 512], BF16, tag="qro")
                qrov = qro.rearrange("p (h i two) -> p h i two", h=8, two=2)
                t1v = tmp1.rearrange("p h (i two) -> p h i two", two=2)
                t2v = tmp2.rearrange("p h (i two) -> p h i two", two=2)
                nc.vector.tensor_sub(qrov[:, :, :, 0], t1v[:, :, :, 0],
                                     t2v[:, :, :, 1])
                nc.gpsimd.tensor_add(qrov[:, :, :, 1], t1v[:, :, :, 1],
                                     t2v[:, :, :, 0])
                bc = bc_q if is_q else bc_k
                if bc is not None:
                    nc.vector.tensor_add(qro, qro,
                                         bc[:, t].rearrange("p h d -> p (h d)"))
                # --- transpose head-pairs -> qT/kT ---
                dstT = qT if is_q else kT
                tp = ps_tp.tile([P, 4, P], BF16, tag="tp")
                for hp in range(4):
                    nc.tensor.transpose(tp[:, hp, :], qro[:, hp * 128:(hp + 1) * 128],
                                        ident)
                if os.environ.get("KSPLITCP"):
                    for hp in range(4):
                        nc.vector.tensor_copy(
                            dstT[:, hs // 2 + hp, t * 128:(t + 1) * 128],
                            tp[:, hp, :])
                else:
                    nc.vector.tensor_copy(
                        dstT[:, hs // 2:hs // 2 + 4, t * 128:(t + 1) * 128], tp)

        # interleave nch pairs so each LN chain's latency hides behind the
        # other stream's matmuls on the PE
        for na, nb in [(0, 2), (4, 1), (3, 5)]:
            wta = emit_wblock(na)
            wtb = emit_wblock(nb)
            for t in range(NTT):
                emit_chunk(na, t, wta)
                emit_chunk(nb, t, wtb)

        # ---------------- proj_w -> projT (tail of phase 2) ----------------
        projT = arena.tile([P, 8, C], BF16, tag="xT")
        for j in ([] if os.environ.get("KSUBX") else range(8)):
            wl2 = wld.tile([P, C], F32, tag="wl")
            nc.sync.dma_start(out=wl2, in_=PW[j * 128:(j + 1) * 128, :])
            wl2b = wld.tile([P, C], BF16, tag="wlb")
            nc.scalar.copy(wl2b, wl2)
            for half in range(2):
                tp = ps_tp.tile([P, 4, P], BF16, tag="tp")
                for jj in range(4):
                    cc = half * 4 + jj
                    nc.tensor.transpose(tp[:, jj, :],
                                        wl2b[:, cc * 128:(cc + 1) * 128], ident)
                nc.vector.tensor_copy(
                    projT[:, half * 4:half * 4 + 4, j * 128:(j + 1) * 128], tp)

    ph12.close()

    if NPH < 4:
        return

    if os.environ.get("KSDPA") == "base":
        # baseline-style SDPA (known-good on HW)
        QCHB = [(0, 512), (512, 512), (897, 128)]
        ones_col_r = ones_col
        with tc.tile_pool(name="expp", bufs=10) as expp, \
             tc.tile_pool(name="sml", bufs=4) as sml, \
             tc.tile_pool(name="psq", bufs=4, space="PSUM") as psq, \
             tc.tile_pool(name="pso", bufs=2, space="PSUM") as psop, \
             tc.tile_pool(name="psb", bufs=2, space="PSUM") as psbp:
            for h in range(H):
                g, half = h // 2, (h % 2) * 64
                ex_tiles = []
                for kt in range(NTT):
                    rw = 128 if kt < 8 else 1
                    ex = expp.tile([P, NT], BF16, tag="expS")
                    ex_tiles.append(ex)
                    for (q0, qn) in QCHB:
                        pss = psq.tile([P, 512], F32, tag="ps")
                        nc.tensor.matmul(
                            pss[:rw, :qn],
                            kT[half:half + 64, g, kt * 128:kt * 128 + rw],
                            qT[half:half + 64, g, q0:q0 + qn],
                            start=True, stop=True)
                        nc.scalar.activation(ex[:rw, q0:q0 + qn], pss[:rw, :qn],
                                             AF.Exp, scale=SCALE)
                for (q0, qn) in QCHB:
                    po = psop.tile([HD + 1, 512], F32, tag="po")
                    for kt in range(NTT):
                        rw = 128 if kt < 8 else 1
                        nc.tensor.matmul(po[:, :qn], vA[:rw, kt, h, :],
                                         ex_tiles[kt][:rw, q0:q0 + qn],
                                         start=(kt == 0), stop=(kt == 8))
                    rs = sml.tile([1, 512], F32, tag="rs")
                    if os.environ.get("KRECIP") == "approx":
                        rs0 = sml.tile([1, 512], F32, tag="rs0")
                        nc.vector.reciprocal_approx_fast(rs0[:, :qn],
                                                         po[HD:HD + 1, :qn])
                        with nc.allow_low_precision(reason="softmax denom"):
                            nc.vector.tensor_copy(_r(rs[:, :qn]), rs0[:, :qn])
                    else:
                        with nc.allow_low_precision(reason="softmax denom"):
                            nc.vector.reciprocal(_r(rs[:, :qn]), po[HD:HD + 1, :qn])
                    pb = psbp.tile([HD, 512], F32, tag="pb")
                    nc.tensor.matmul(pb[:, :qn], _r(ones_col), _r(rs[:, :qn]),
                                     start=True, stop=True)
                    nc.vector.tensor_copy(oT[half:half + 64, g, q0:q0 + qn],
                                          po[:HD, :qn])
                    nc.vector.tensor_mul(oT[half:half + 64, g, q0:q0 + qn],
                                         oT[half:half + 64, g, q0:q0 + qn],
                                         pb[:, :qn])
        _run_phase5 = True
    else:
        _run_phase5 = False

    if _run_phase5:
        pass
    elif True:
        pass
    # ---------------- phase 4: SDPA ----------------
    if os.environ.get("KSDPA") == "base":
        pass
    else:
     with tc.tile_pool(name="expp", bufs=20) as expp, \
         tc.tile_pool(name="rsp", bufs=3) as rsp, \
         tc.tile_pool(name="rsbp", bufs=1) as rsbp, \
         tc.tile_pool(name="ps_qs", bufs=2, space="PSUM") as ps_qs, \
         tc.tile_pool(name="ps_po", bufs=2, space="PSUM") as ps_po:
        ex_store = {}

        def emit_qk(h):
            g, half = h // 2, (h % 2) * 64
            ex_store[h] = []
            for kt in range(NTT):
                qs = ps_qs.tile([P, NTOK], F32, tag="qs")
                lsrc, rsrc = (kT, qT) if not os.environ.get("KSUBX") else (xT, xT)
                for (q0, qn, pc) in QCH:
                    nc.tensor.matmul(
                        qs[:, pc:pc + qn],
                        lsrc[half:half + 64, g, kt * 128:(kt + 1) * 128],
                        rsrc[half:half + 64, g, q0:q0 + qn],
                        start=True, stop=True)
                ex = expp.tile([P, NTOK], BF16, tag="ex")
                nc.scalar.activation(ex, qs, AF.Exp, scale=SCALE)
                ex_store[h].append(ex)
                if os.environ.get("KDBG2") and h == 0 and kt == 0:
                    dbga = arena.tile([P, 1024], F32, tag="dbga")
                    nc.vector.tensor_copy(dbga, ex[:, 0:1024])
                    nc.sync.dma_start(out=OUT[0:128, :], in_=dbga)
                    dbgs = arena.tile([P, 1024], F32, tag="dbgs")
                    nc.vector.tensor_copy(dbgs, qs[:, 0:1024])
                    nc.sync.dma_start(out=OUT[640:768, :], in_=dbgs)

        def emit_pv(h):
            # fast drain: copy v-part -> oT (unnormalized) and sums row ->
            # sums_all[h]; normalization deferred to emit_norm.
            g, half = h // 2, (h % 2) * 64
            exs = ex_store.pop(h)
            pos = []
            for ci, (q0, qn, pc) in enumerate(QCH):
                po = ps_po.tile([P, 512], F32, tag="po")
                for kt in range(NTT):
                    nc.tensor.matmul(po[0:HD + 1, 0:qn], vA[:, kt, h, :],
                                     exs[kt][:, pc:pc + qn],
                                     start=(kt == 0), stop=(kt == 8))
                pos.append(po)
                if ci == 0:
                    continue
                _drain_chunk(h, ci - 1, pos[ci - 1])
            _drain_chunk(h, 2, pos[2])

        sums_store = {}

        def _drain_chunk(h, ci, po):
            g, half = h // 2, (h % 2) * 64
            q0, qn, pc = QCH[ci]
            nc.vector.tensor_copy(oT[half:half + 64, g, q0:q0 + qn],
                                  po[0:HD, 0:qn])
            if ci == 0:
                sums = rsp.tile([1, NTOK], F32, tag="sums")
                nc.vector.memset(sums[:, 1024:NTOK], 1.0)
                sums_store[h] = sums
            nc.vector.tensor_copy(sums_store[h][:, q0:q0 + qn],
                                  po[HD:HD + 1, 0:qn])

        def emit_norm(h):
            # batched per-head normalization: one [1,1152] recip + one
            # broadcast + one in-place mult over the whole head
            g, half = h // 2, (h % 2) * 64
            sums = sums_store.pop(h)
            rs = rsp.tile([1, NTOK], F32, tag="rs")
            nc.vector.reciprocal_approx_fast(rs, sums)
            rsb = rsbp.tile([P, NTOK], F32, tag="rsb")
            nc.gpsimd.partition_broadcast(rsb, rs, channels=P)
            dst = oT[half:half + 64, g, :]
            nc.vector.tensor_mul(dst, rsb[half:half + HD, :], dst)

        emit_qk(0)
        prev = []
        for h in range(H):
            if h + 1 < H:
                emit_qk(h + 1)
            emit_pv(h)
            if prev:
                emit_norm(prev.pop())
            prev.append(h)
        emit_norm(prev.pop())

    if os.environ.get("KDBG"):
        dbg = arena.tile([P, 1024], F32, tag="dbg")
        nc.vector.tensor_copy(dbg, qT[:, 0, 0:1024])
        nc.sync.dma_start(out=OUT[0:128, :], in_=dbg)
        dbg2 = arena.tile([P, 1024], F32, tag="dbg2")
        nc.vector.tensor_copy(dbg2, kT[:, 0, 0:1024])
        nc.sync.dma_start(out=OUT[128:256, :], in_=dbg2)
        dbg3 = arena.tile([P, 1024], F32, tag="dbg3")
        nc.vector.tensor_copy(dbg3[:, 0:975],
                              vA[:, 0, 0:15, :].rearrange("p h d -> p (h d)"))
        nc.sync.dma_start(out=OUT[256:384, :], in_=dbg3)
        dbg4 = arena.tile([P, 1024], F32, tag="dbg4")
        nc.vector.tensor_copy(dbg4, oT[:, 0, 0:1024])
        nc.sync.dma_start(out=OUT[384:512, :], in_=dbg4)
        dbg5 = arena.tile([P, 1024], F32, tag="dbg5")
        nc.vector.tensor_copy(dbg5, projT[:, 0, 0:1024])
        nc.sync.dma_start(out=OUT[512:640, :], in_=dbg5)
        return

    if NPH < 5:
        return
    # ---------------- phase 5: proj ----------------
    with tc.tile_pool(name="ps_py", bufs=4, space="PSUM") as ps_py, \
         tc.tile_pool(name="yp", bufs=4) as yp:
        for t in range(NTT):
            rw = 1 if t == 8 else P
            for n2 in range(2):
                py = ps_py.tile([P, 512], F32, tag="py")
                for cc in range(8):
                    nc.tensor.matmul(py[:rw], oT[:, cc, t * 128:t * 128 + rw],
                                     projT[:, cc, n2 * 512:(n2 + 1) * 512],
                                     start=(cc == 0), stop=(cc == 7 and not has_pj_b))
                if has_pj_b:
                    nc.tensor.matmul(py[:rw], _r(ones_row[:, :rw]),
                                     _r(pb_row[:, n2 * 512:(n2 + 1) * 512]),
                                     start=False, stop=True)
                ysb = yp.tile([P, 512], F32, tag="ysb")
                nc.vector.tensor_copy(ysb[:rw], py[:rw])
                nc.sync.dma_start(
                    out=OUT[t * 128:t * 128 + rw, n2 * 512:(n2 + 1) * 512],
                    in_=ysb[:rw])


_NC_CACHE = {}


def _build_nc(flags):
    if flags in _NC_CACHE:
        return _NC_CACHE[flags]
    nc = _bacc.Bacc()
    X = nc.declare_dram_parameter("x", [NT, C], F32, isOutput=False)
    ROPE = nc.declare_dram_parameter("rope", [NT - 1, 2 * HD], F32, isOutput=False)
    QKVW = nc.declare_dram_parameter("qkv_w", [3 * C, C], F32, isOutput=False)
    QKVB = nc.declare_dram_parameter("qkv_b", [3 * C], F32, isOutput=False)
    QNW = nc.declare_dram_parameter("qn_w", [HD], F32, isOutput=False)
    QNB = nc.declare_dram_parameter("qn_b", [HD], F32, isOutput=False)
    KNW = nc.declare_dram_parameter("kn_w", [HD], F32, isOutput=False)
    KNB = nc.declare_dram_parameter("kn_b", [HD], F32, isOutput=False)
    PW = nc.declare_dram_parameter("proj_w", [C, C], F32, isOutput=False)
    PB = nc.declare_dram_parameter("proj_b", [C], F32, isOutput=False)
    OUT = nc.declare_dram_parameter("out", [NT, C], F32, isOutput=True)
    with ExitStack() as ctx:
        tc = ctx.enter_context(tile.TileContext(nc))
        build_kernel(ctx, tc, X, ROPE, QKVW, QKVB, QNW, QNB, KNW, KNB, PW, PB,
                     OUT, *flags)
    nc.finalize()
    _NC_CACHE[flags] = nc
    return nc


def kernel(x, rope, qkv_w, qkv_b, qn_w, qn_b, kn_w, kn_b, proj_w, proj_b):
    global LAST_RESULT
    flags = (bool(np.any(qkv_b)), bool(np.any(qn_b)), bool(np.any(kn_b)),
             bool(np.any(proj_b)))
    nc = _build_nc(flags)
    shared = dict(rope=np.asarray(rope, np.float32),
                  qkv_w=np.asarray(qkv_w, np.float32),
                  qkv_b=np.asarray(qkv_b, np.float32),
                  qn_w=np.asarray(qn_w, np.float32),
                  qn_b=np.asarray(qn_b, np.float32),
                  kn_w=np.asarray(kn_w, np.float32),
                  kn_b=np.asarray(kn_b, np.float32),
                  proj_w=np.asarray(proj_w, np.float32),
                  proj_b=np.asarray(proj_b, np.float32))
    x = np.asarray(x, np.float32)
    in_maps = [dict(x=np.ascontiguousarray(x[i]), **shared) for i in range(B)]
    res = run_bass_kernel_spmd(nc, in_maps, list(range(B)))
    LAST_RESULT = res
    return np.stack([res.results[i]["out"] for i in range(B)], axis=0)
